# revision 14
# baseline (speedup 1.0000x reference)
"""Self-contained Trainium2 Bass kernel for a post-LN transformer block.

Problem: y = LN(h + MLP(h)), h = LN(x + CausalAttn(x)), B=2, L=2048, D=1024,
H=16 heads, MLP hidden 4096, shared LN params, exact GELU, fp32 I/O.

Sharding (8 cores): core c handles batch b=c//4, head-group q=c%4 (heads
4q..4q+3) for attention, then rows [512q, 512q+512) of batch b for the
MLP/LN part. One 8-core AllToAll per half-row round re-shards from
column(head)-split to row-split (other-batch slots are zeroed via zmask so
receivers just add both batch halves). x arrives host-pre-transposed (xT).

Schedule: chunks are processed largest-first within each round (even round
2,4,6,0; odd round 7,5,3,1) so the serial exp tail of the LAST chunk is
minimal and the collective triggers as early as possible. K projection runs
in 256-token substeps just-in-time; V pairs are emitted inside the chunk
that first needs them. The whole normalize->zmask-mul->send-DMA->trigger
path is high-priority (zmask muls on the otherwise idle GpSimd engine) so
the Tile scheduler cannot delay a round's sends behind recv/LN work. MLP:
m1 in row-halves (half 0 only needs round A); m2 in tb-pairs with w2
half-resident in SBUF (h4 0-3 resident in slots freed after attention,
h4 4-7 streamed once per pair) so the two tbs of a pair finish staggered
and the final LN/DMA tail is short. Matmuls in bf16 with fp32 PSUM
accumulation; residuals/LN in fp32.
"""

import contextlib
import ctypes
import sys
import types

import numpy as np

B, L, D = 2, 2048, 1024
H, HD = 16, 64
DFF = 4 * D
EPS = 1e-5
NCORES = 8
ROWS = L // 4  # 512 rows per core for MLP phase
HPC = 4  # heads per core
HCOLS = HPC * HD  # 256 attn-out cols per core
NTB = L // 128  # 16 token blocks per batch
NRB = ROWS // 128  # 4 token blocks per core row-slice
NJ2 = L // 256  # 8 query chunks of 256


def _install_axon_hooks_shim():
    """Provide antenv.axon_hooks (NTFF profiling hook) when the image lacks it.

    Needed only when profiling (BASS_TRACE=1); harmless otherwise.
    """
    try:
        from antenv.axon_hooks import get_axon_ntff_profile_hook  # noqa: F401

        return
    except ImportError:
        pass
    try:
        import antenv
    except ImportError:
        return

    mod = types.ModuleType("antenv.axon_hooks")
    _state = {"hook": None}
    mod.set_axon_ntff_profile_hook = lambda h: _state.__setitem__("hook", h)
    mod.get_axon_ntff_profile_hook = lambda: _state["hook"]
    sys.modules["antenv.axon_hooks"] = mod
    antenv.axon_hooks = mod

    try:
        lib = ctypes.CDLL("/opt/axon/libaxon_pjrt.so")
    except OSError:
        return
    if not hasattr(lib, "axon_start_nrt_profile"):
        return
    lib.axon_start_nrt_profile.argtypes = [
        ctypes.POINTER(ctypes.c_int64),
        ctypes.c_size_t,
    ]
    lib.axon_start_nrt_profile.restype = ctypes.c_int64
    lib.axon_stop_nrt_profile.argtypes = [ctypes.c_char_p]
    lib.axon_stop_nrt_profile.restype = ctypes.c_int64

    @contextlib.contextmanager
    def _hook(output_dir, device_ids):
        import jax

        jax.devices()
        if device_ids:
            ids = (ctypes.c_int64 * len(device_ids))(*device_ids)
            rc = lib.axon_start_nrt_profile(ids, len(device_ids))
        else:
            rc = lib.axon_start_nrt_profile(None, 0)
        if rc != 0:
            raise RuntimeError(f"axon_start_nrt_profile rc={rc}")
        try:
            yield
        finally:
            n = lib.axon_stop_nrt_profile(str(output_dir).encode())
            print(f"profile: {n} file(s) -> {output_dir}", file=sys.stderr)

    mod.set_axon_ntff_profile_hook(_hook)


_install_axon_hooks_shim()

import concourse.bass as bass  # noqa: E402
import concourse.tile as tile  # noqa: E402
from concourse import bacc, mybir  # noqa: E402
from concourse.bass_utils import run_bass_kernel_spmd  # noqa: E402
from concourse.masks import make_identity  # noqa: E402

F32 = mybir.dt.float32
BF16 = mybir.dt.bfloat16

EVEN_ORDER = (2, 4, 6, 0)
ODD_ORDER = (7, 5, 3, 1)


def _build():
    nc = bacc.Bacc(
        "TRN2", target_bir_lowering=False, debug=False, num_devices=NCORES
    )

    def din(name, shape, dt=F32):
        return nc.dram_tensor(name, shape, dt, kind="ExternalInput").ap()

    # All large inputs are host-pre-arranged partition-major so every DMA
    # line is a long contiguous run (max descriptor efficiency).
    xbT = din("xbT", [128, 4, 8, 512], BF16)  # x[b].T as [p, tq, ic, tok]
    xr = din("xr", [ROWS, D], F32)  # this core's row slice of x, fp32
    wq_c = din("wq_c", [128, 8, HCOLS], BF16)  # [p, ic, col], pre-scaled 1/8
    wk_c = din("wk_c", [128, 8, HCOLS], BF16)
    wv_c = din("wv_c", [128, 8, HCOLS], BF16)
    w1 = din("w1", [128, 8, 8, 512], BF16)  # [p, o4, ic, col]
    b1 = din("b1", [DFF])
    w2 = din("w2", [128, 8, 4, D], BF16)  # [p, h4, hs, col]
    mask_tri = din("mask_tri", [128, 128])  # 1 where k<=q else 0
    zmask = din("zmask", [NCORES])  # 1 for same-batch a2a slots else 0
    out = nc.dram_tensor("out", [ROWS, D], F32, kind="ExternalOutput").ap()

    with tile.TileContext(nc) as tc, contextlib.ExitStack() as ctx:
        pb = ctx.enter_context(tc.tile_pool(name="pb", bufs=1))  # persistent
        pc = ctx.enter_context(tc.tile_pool(name="pc", bufs=1))  # constants
        pw = ctx.enter_context(tc.tile_pool(name="pw", bufs=1))  # resident W
        pws = ctx.enter_context(tc.tile_pool(name="pws", bufs=3))  # streamed W
        ps = ctx.enter_context(tc.tile_pool(name="ps", bufs=3))  # small tiles
        pr = ctx.enter_context(tc.tile_pool(name="pr", bufs=3))  # recv tiles
        pe = ctx.enter_context(tc.tile_pool(name="pe", bufs=4))  # exp tiles
        pp = ctx.enter_context(tc.tile_pool(name="pp", bufs=2, space="PSUM"))
        pd = ctx.enter_context(tc.tile_pool(name="pd", bufs=1, space="DRAM"))

        # ---- big SBUF tiles (tag-shared slots; lifetimes disjoint) ----
        xT = pb.tile([128, 4, 8, 512], BF16, tag="slotA")  # [p, tq, ic, tok]
        KT = pb.tile([128, 2, L], BF16, tag="slotC")  # dead after last scores
        QT = pb.tile([128, 2, L], BF16, tag="slotD")  # dead after last scores
        V_ext = pb.tile([128, NTB, HPC, HD + 1], BF16, tag="slotE")
        attn_sb = pb.tile([128, NTB, HCOLS], BF16, tag="slotF")
        res1 = pb.tile([128, NRB, D], F32, tag="slotG")
        hT = pb.tile([128, 8, ROWS], BF16, tag="slotH")

        # ---- startup DMAs, most-urgent first: wk + first xT half gate the
        #      first K substep; wq/wv go via the gpsimd queue in parallel ----
        # Spread across 4 DMA queues (sync/gpsimd/scalar/vector) so the
        # early transfers run in parallel instead of serializing on one
        # queue (~155 GB/s per queue).
        wk_sb = pw.tile([128, 8, HCOLS], BF16)
        nc.sync.dma_start(out=wk_sb, in_=wk_c[:, :, :])
        nc.sync.dma_start(out=xT[:, 0, :, 0:256], in_=xbT[:, 0, :, 0:256])
        wq_sb = pw.tile([128, 8, HCOLS], BF16)
        nc.gpsimd.dma_start(out=wq_sb, in_=wq_c[:, :, :])
        nc.scalar.dma_start(out=xT[:, 1, :, :], in_=xbT[:, 1, :, :])
        nc.sync.dma_start(out=xT[:, 0, :, 256:512], in_=xbT[:, 0, :, 256:512])
        wv_sb = pw.tile([128, 8, HCOLS], BF16)
        nc.gpsimd.dma_start(out=wv_sb, in_=wv_c[:, :, :])
        nc.scalar.dma_start(out=xT[:, 2, :, :], in_=xbT[:, 2, :, :])
        nc.gpsimd.dma_start(out=xT[:, 3, :, :], in_=xbT[:, 3, :, :])

        # ---- constants ----
        ident_f = pc.tile([128, 128], F32)
        make_identity(nc, ident_f)
        ident_b = pc.tile([128, 128], BF16)
        make_identity(nc, ident_b)
        mask_sb = pc.tile([128, 128], BF16)
        nc.gpsimd.dma_start(out=mask_sb, in_=mask_tri[:, :])
        eps_sb = pc.tile([128, 1], F32)
        nc.vector.memset(eps_sb, EPS)
        b1_sb = pc.tile([128, 32], F32)  # per-partition bias for m1^T chunks
        nc.gpsimd.dma_start(
            out=b1_sb,
            in_=bass.AP(tensor=b1.tensor, offset=b1.offset, ap=[[1, 128], [128, 32]]),
        )
        zm_sb = pc.tile([128, NCORES], F32)
        nc.gpsimd.dma_start(
            out=zm_sb,
            in_=bass.AP(
                tensor=zmask.tensor, offset=zmask.offset, ap=[[0, 128], [1, NCORES]]
            ),
        )

        # ---- a2a DRAM buffers (bf16 payload, two half-row rounds; senders
        #      zero their payload toward other-batch receivers via zmask) ----
        a2a_in1 = pd.tile([NCORES, ROWS // 2, HCOLS], BF16)
        a2a_out1 = pd.tile([NCORES, ROWS // 2, HCOLS], BF16)
        a2a_in2 = pd.tile([NCORES, ROWS // 2, HCOLS], BF16)
        a2a_out2 = pd.tile([NCORES, ROWS // 2, HCOLS], BF16)

        # residual base for MLP rows arrives in the background
        nc.scalar.dma_start(out=res1, in_=xr.rearrange("(t p) c -> p t c", p=128))

        nc.vector.memset(V_ext[:, :, :, HD : HD + 1], 1.0)

        # ---- attention building blocks ----
        def q_slice(h, J2):
            p0 = 64 * (h % 2)
            return QT[p0 : p0 + 64, h // 2, J2 * 256 : (J2 + 1) * 256]

        def k_slice(h, kb):
            p0 = 64 * (h % 2)
            return KT[p0 : p0 + 64, h // 2, kb * 128 : (kb + 1) * 128]

        def k_sub(t):
            # K projection for 256-token substep t (tokens 256t..256t+256)
            psk = pp.tile([128, 2, 256], F32, tag="ps", name=f"psk_{t}")
            for oc in range(2):
                for ic in range(8):
                    nc.tensor.matmul(
                        psk[:, oc, :],
                        wk_sb[:, ic, oc * 128 : (oc + 1) * 128],
                        xT[:, t // 2, ic, (t % 2) * 256 : (t % 2) * 256 + 256],
                        start=(ic == 0),
                        stop=(ic == 7),
                    )
            nc.vector.tensor_copy(KT[:, :, t * 256 : (t + 1) * 256], psk)

        def q_proj(J2):
            tq, th = J2 // 2, (J2 % 2) * 256
            psq = pp.tile([128, 2, 256], F32, tag="pqv", name=f"psq_{J2}")
            for oc in range(2):
                for ic in range(8):
                    nc.tensor.matmul(
                        psq[:, oc, :],
                        wq_sb[:, ic, oc * 128 : (oc + 1) * 128],
                        xT[:, tq, ic, th : th + 256],
                        start=(ic == 0),
                        stop=(ic == 7),
                    )
            nc.vector.tensor_copy(QT[:, :, J2 * 256 : (J2 + 1) * 256], psq)

        def v_pair(tb2):
            psv = pp.tile([128, 2, 256], F32, tag="pqv", name=f"psv_{tb2}")
            for kk in range(2):
                tb = tb2 + kk
                for ic in range(8):
                    nc.tensor.matmul(
                        psv[:, kk, :],
                        xT[:, tb // 4, ic, (tb % 4) * 128 : (tb % 4) * 128 + 128],
                        wv_sb[:, ic, :],
                        start=(ic == 0),
                        stop=(ic == 7),
                    )
            nc.vector.tensor_copy(
                V_ext[:, tb2 : tb2 + 2, :, 0:HD],
                psv.rearrange("p k (h d) -> p k h d", h=HPC),
            )

        def process_chunk(J2, ain, hooks=None):
            """Scores -> exp -> AV -> normalize -> a2a sends for one 256-query
            chunk. hooks[kp] emits prerequisite K substeps / V pairs."""
            for hp in range(2):
                h0, h1 = 2 * hp, 2 * hp + 1
                psu = pp.tile(
                    [128, 2, 2, HD + 1], F32, tag="pu", name=f"psu_{J2}_{hp}"
                )
                exps = [None] * (J2 + 1)

                def av_quad(kp, J2=J2, hp=hp, psu=psu, exps=exps):
                    # psu packs 4 accumulation regions (hh, js) in ONE psum
                    # bank. start=True marks the WHOLE bank pending-zero, so
                    # only the very first matmul into the bank may carry it:
                    # each region's first write then consumes its pending
                    # bytes (overwrite), later writes accumulate.
                    expP = exps[kp]
                    for idx in range(4):
                        hh = idx // 2  # 0 -> h0, 1 -> h1
                        kb = 2 * kp + (idx % 2)
                        hg = 2 * hp + hh
                        for js in range(2):
                            if 2 * J2 + js < kb:
                                continue
                            nc.tensor.matmul(
                                psu[:, hh, js, :],
                                expP[:, idx, js * 128 : (js + 1) * 128],
                                V_ext[:, kb, hg, :],
                                start=(kb == 0 and idx == 0 and js == 0),
                                stop=(kb == 2 * J2 + js),
                            )

                for kp in range(J2 + 1):
                    if hp == 0 and hooks and kp in hooks:
                        for fn in hooks[kp]:
                            fn()
                    k0, k1 = 2 * kp, 2 * kp + 1
                    pssP = pp.tile(
                        [128, 4, 256], F32, tag="ps", name=f"pssP_{J2}_{hp}_{kp}"
                    )
                    # bank0 <- head h0 (rows 0-63), bank1 <- head h1 (rows
                    # 64-127); pairs target disjoint row groups + banks so
                    # they run concurrently in the PE array.
                    nc.tensor.matmul(
                        pssP[:, 0, :], k_slice(h0, k0), q_slice(h0, J2),
                        start=True, stop=True,
                    )
                    nc.tensor.matmul(
                        pssP[:, 2, :], k_slice(h1, k0), q_slice(h1, J2),
                        start=True, stop=True,
                    )
                    nc.tensor.matmul(
                        pssP[:, 1, :], k_slice(h0, k1), q_slice(h0, J2),
                        start=True, stop=True,
                    )
                    nc.tensor.matmul(
                        pssP[:, 3, :], k_slice(h1, k1), q_slice(h1, J2),
                        start=True, stop=True,
                    )
                    expP = pe.tile([128, 4, 256], BF16, tag="expT",
                                   name=f"expP_{J2}_{hp}_{kp}")
                    nc.scalar.activation(
                        expP, pssP, mybir.ActivationFunctionType.Exp
                    )
                    if kp == J2:  # diagonal pair: causal mask inside
                        for idx, js in ((0, 0), (1, 1), (2, 0), (3, 1)):
                            nc.vector.tensor_mul(
                                expP[:, idx, js * 128 : (js + 1) * 128],
                                expP[:, idx, js * 128 : (js + 1) * 128],
                                mask_sb,
                            )
                    exps[kp] = expP
                    # 2-unit lookahead: av_quad(kp-2) consumes an exp that
                    # has had two scores-units (~1.1us) of PE time to finish,
                    # so the PE never stalls on the ~0.85us exp latency.
                    if kp >= 2:
                        av_quad(kp - 2)
                if J2 >= 1:
                    av_quad(J2 - 1)
                av_quad(J2)
                # softmax normalize + write attn_sb columns for this pair.
                # High priority: the sends (and so the collective trigger)
                # depend on these; don't let the scheduler defer them.
                with tc.high_priority():
                    for hh in range(2):
                        hg = 2 * hp + hh
                        for js in range(2):
                            rec = ps.tile([128, 1], F32, tag="rec")
                            nc.vector.reciprocal(
                                rec, psu[:, hh, js, HD : HD + 1]
                            )
                            nc.vector.tensor_scalar_mul(
                                attn_sb[:, 2 * J2 + js, hg * HD : (hg + 1) * HD],
                                psu[:, hh, js, 0:HD],
                                rec,
                            )
            # ship this chunk's two token blocks to both batch slots (the
            # other-batch copy is zeroed so receivers just add both). All
            # high priority so the round's trigger fires as soon as possible
            # (gpsimd is unusable here: its TENSOR_SCALAR is ~8us/op).
            with tc.high_priority():
                for s in (J2 // 2, 4 + J2 // 2):
                    st = pr.tile(
                        [128, 2, HCOLS], BF16, tag="st", name=f"st_{J2}_{s}"
                    )
                    nc.vector.tensor_scalar_mul(
                        st, attn_sb[:, 2 * J2 : 2 * J2 + 2, :], zm_sb[:, s : s + 1]
                    )
                    nc.sync.dma_start(
                        out=ain[s].rearrange("(t p) c -> p t c", p=128), in_=st
                    )

        # ---- attention: biggest chunk of each round first so the round's
        #      last (smallest) chunk has a tiny exp tail and the collective
        #      triggers right after the round's PE work ends ----
        k_sub(0)
        q_proj(2)
        v_pair(0)
        process_chunk(2, a2a_in1, hooks={
            1: [lambda: k_sub(1), lambda: v_pair(2)],
            2: [lambda: k_sub(2), lambda: v_pair(4)],
        })
        q_proj(4)
        process_chunk(4, a2a_in1, hooks={
            3: [lambda: k_sub(3), lambda: v_pair(6)],
            4: [lambda: k_sub(4), lambda: v_pair(8)],
        })
        q_proj(6)
        process_chunk(6, a2a_in1, hooks={
            5: [lambda: k_sub(5), lambda: v_pair(10)],
            6: [lambda: k_sub(6), lambda: v_pair(12)],
        })
        q_proj(0)
        process_chunk(0, a2a_in1)
        with tc.high_priority():
            nc.gpsimd.collective_compute(
                "AllToAll",
                mybir.AluOpType.bypass,
                replica_groups=[list(range(NCORES))],
                ins=[a2a_in1[:]],
                outs=[a2a_out1[:]],
            )

        k_sub(7)
        v_pair(14)
        q_proj(7)
        process_chunk(7, a2a_in2)
        q_proj(5)
        process_chunk(5, a2a_in2)
        q_proj(3)
        process_chunk(3, a2a_in2)
        q_proj(1)
        process_chunk(1, a2a_in2)
        with tc.high_priority():
            nc.gpsimd.collective_compute(
                "AllToAll",
                mybir.AluOpType.bypass,
                replica_groups=[list(range(NCORES))],
                ins=[a2a_in2[:]],
                outs=[a2a_out2[:]],
            )

        # ---- resident slice of w2 (h4 0-1) into the SBUF slots that die
        #      with the attention phase (exact-size fits, no slot growth);
        #      h4 2-7 are streamed per m2 pair ----
        w2resA = pb.tile([128, 4, D], BF16, tag="slotE")
        nc.sync.dma_start(out=w2resA, in_=w2[:, 0, :, :])
        w2resB = pb.tile([128, 4, D], BF16, tag="slotF")
        nc.sync.dma_start(out=w2resB, in_=w2[:, 1, :, :])

        # ---- recv + LN1 + transpose to hT, then m1 in token halves so the
        #      round-A half starts while round B's collective drains ----
        h_sb = pb.tile([128, NRB, D], F32, tag="slotD")  # reuses QT slot
        h_bf = pb.tile([128, NRB, D], BF16, tag="slotI")  # bf16 copy for hT
        res2 = pb.tile([128, NRB, D], F32, tag="slotC")  # reuses KT slot
        gT = pb.tile([128, 32, ROWS], BF16, tag="slotA")  # reuses xT slot

        def recv_adds(tb, aout, ti):
            # sync-issued DMAs (collective-completion deps enforced there),
            # emitted only after all a2a sends so those never block; adds
            # split gpsimd/vector by column group (disjoint res1 ranges) so
            # each token block's chain runs on two engines concurrently and
            # the vector share stays small (it also carries LN1 + the odd
            # round's normalize in this window)
            for g in range(4):
                eng = nc.gpsimd if g < 2 else nc.vector
                r0 = pr.tile([128, HCOLS], BF16, tag="r0", name=f"r0_{tb}_{g}")
                nc.sync.dma_start(
                    out=r0,
                    in_=aout[g].rearrange("(t p) c -> p t c", p=128)[:, ti, :],
                )
                r1 = pr.tile([128, HCOLS], BF16, tag="r1", name=f"r1_{tb}_{g}")
                nc.sync.dma_start(
                    out=r1,
                    in_=aout[4 + g].rearrange("(t p) c -> p t c", p=128)[
                        :, ti, :
                    ],
                )
                # exactly one of the pair is nonzero (zmask), so the bf16
                # intermediate sum is exact
                ta = pr.tile([128, HCOLS], BF16, tag="ta", name=f"ta_{tb}_{g}")
                eng.tensor_add(ta, r0, r1)
                dst = res1[:, tb, g * HCOLS : (g + 1) * HCOLS]
                eng.tensor_add(dst, dst, ta)

        def ln_row(src_t, tb, out_ap, bf_ap=None):
            stats = ps.tile([128, 2, 6], F32, tag="stats")
            nc.vector.bn_stats(stats[:, 0, :], src_t[:, tb, 0:512])
            nc.vector.bn_stats(stats[:, 1, :], src_t[:, tb, 512:1024])
            mv = ps.tile([128, 2], F32, tag="mv")
            nc.vector.bn_aggr(mv, stats)
            std = ps.tile([128, 1], F32, tag="std")
            nc.scalar.activation(
                std, mv[:, 1:2], mybir.ActivationFunctionType.Sqrt,
                bias=eps_sb[:, 0:1], scale=1.0,
            )
            rstd = ps.tile([128, 1], F32, tag="rstd")
            nc.vector.reciprocal(rstd, std)
            # ln_g == 1, ln_b == 0 in this problem, so affine is identity
            nc.vector.tensor_scalar(
                out=out_ap,
                in0=src_t[:, tb, :],
                scalar1=mv[:, 0:1],
                scalar2=rstd,
                op0=mybir.AluOpType.subtract,
                op1=mybir.AluOpType.mult,
            )
            if bf_ap is not None:
                # bf16 shadow copy on the (idle-here) scalar engine
                nc.scalar.copy(bf_ap, out_ap)

        def ln_hT(tb):
            ln_row(res1, tb, h_sb[:, tb, :], h_bf[:, tb, :])
            for f4 in range(2):
                psT = pp.tile(
                    [128, 4, 128], BF16, tag="pu", name=f"psT_{tb}_{f4}"
                )
                for fs in range(4):
                    fc = 4 * f4 + fs
                    nc.tensor.transpose(
                        psT[:, fs, :],
                        h_bf[:, tb, fc * 128 : (fc + 1) * 128],
                        ident_b,
                    )
                nc.vector.tensor_copy(
                    hT[:, 4 * f4 : 4 * f4 + 4, tb * 128 : (tb + 1) * 128],
                    psT,
                )

        def m1_half(half):
            c0 = 256 * half
            for o4 in range(8):
                w1c = pws.tile(
                    [128, 8, 512], BF16, tag="w1c", name=f"w1c_{half}_{o4}"
                )
                nc.sync.dma_start(out=w1c, in_=w1[:, o4, :, :])
                for os_ in range(4):
                    oc = o4 * 4 + os_
                    psm = pp.tile([128, 256], F32, tag="pqv", name=f"psm_{half}_{oc}")
                    for ic in range(8):
                        nc.tensor.matmul(
                            psm,
                            w1c[:, ic, os_ * 128 : (os_ + 1) * 128],
                            hT[:, ic, c0 : c0 + 256],
                            start=(ic == 0),
                            stop=(ic == 7),
                        )
                    nc.scalar.activation(
                        gT[:, oc, c0 : c0 + 256], psm,
                        mybir.ActivationFunctionType.Gelu,
                        bias=b1_sb[:, oc : oc + 1], scale=1.0,
                    )

        def m2_pair(tbp):
            tbs = (2 * tbp, 2 * tbp + 1)
            pso = {
                tb: pp.tile([128, 2, 512], F32, tag="ps", name=f"pso_{tb}")
                for tb in tbs
            }
            # streamed phase: h4 2..7, each w2c shared by both tbs
            for h4 in range(2, 8):
                w2c = pws.tile(
                    [128, 4, D], BF16, tag="w2c", name=f"w2c_{tbp}_{h4}"
                )
                nc.sync.dma_start(out=w2c, in_=w2[:, h4, :, :])
                for hs in range(4):
                    for tb in tbs:
                        for f2 in range(2):
                            nc.tensor.matmul(
                                pso[tb][:, f2, :],
                                gT[:, 4 * h4 + hs, tb * 128 : (tb + 1) * 128],
                                w2c[:, hs, f2 * 512 : (f2 + 1) * 512],
                                start=(h4 == 2 and hs == 0),
                                stop=False,
                            )
            # resident phase per tb (h4 0..1), staggered so the first tb's
            # evacuate/LN/DMA overlaps the second tb's matmuls
            for tb in tbs:
                for h4 in range(2):
                    wr = w2resA if h4 == 0 else w2resB
                    for hs in range(4):
                        for f2 in range(2):
                            nc.tensor.matmul(
                                pso[tb][:, f2, :],
                                gT[:, 4 * h4 + hs, tb * 128 : (tb + 1) * 128],
                                wr[:, hs, f2 * 512 : (f2 + 1) * 512],
                                start=False,
                                stop=(h4 == 1 and hs == 3),
                            )
                # b2 == 0 in this problem (skipped)
                nc.vector.tensor_add(
                    res2[:, tb, :],
                    pso[tb].rearrange("p a b -> p (a b)"),
                    h_sb[:, tb, :],
                )
                o_t = ps.tile([128, D], F32, tag="o_t", bufs=2)
                ln_row(res2, tb, o_t)
                nc.sync.dma_start(out=out[tb * 128 : (tb + 1) * 128, :], in_=o_t)

        # half 0 (round A rows) first; round B's recv + LN overlap m1/m2
        # of half 0, so the PE never waits on the second collective.
        recv_adds(0, a2a_out1, 0)
        recv_adds(1, a2a_out1, 1)
        ln_hT(0)
        ln_hT(1)
        m1_half(0)
        recv_adds(2, a2a_out2, 0)
        recv_adds(3, a2a_out2, 1)
        ln_hT(2)
        ln_hT(3)
        m2_pair(0)
        m1_half(1)
        m2_pair(1)

    nc.compile()
    return nc


_NC_CACHE = [None]


def kernel(**inputs) -> np.ndarray:
    import ml_dtypes

    x = np.asarray(inputs["x"], np.float32)
    wq = np.asarray(inputs["wq"], np.float32)
    wk = np.asarray(inputs["wk"], np.float32)
    wv = np.asarray(inputs["wv"], np.float32)
    w1 = np.asarray(inputs["w1"], np.float32)
    b1 = np.asarray(inputs["b1"], np.float32)
    w2 = np.asarray(inputs["w2"], np.float32)

    # The kernel folds these away; setup_inputs() constructs them as
    # zeros/ones. Fail loudly if that ever changes.
    for nm in ("bq", "bk", "bv", "b2"):
        if nm in inputs:
            assert not np.any(np.asarray(inputs[nm])), f"{nm} expected zero"
    if "ln_b" in inputs:
        assert not np.any(np.asarray(inputs["ln_b"])), "ln_b expected zero"
    if "ln_g" in inputs:
        assert np.all(np.asarray(inputs["ln_g"]) == 1.0), "ln_g expected ones"

    if _NC_CACHE[0] is None:
        _NC_CACHE[0] = _build()
    nc = _NC_CACHE[0]

    bf = ml_dtypes.bfloat16

    def pmaj_in(m):  # [D, cols] -> [p, ic, cols] partition-major
        return np.ascontiguousarray(
            m.reshape(8, 128, m.shape[1]).transpose(1, 0, 2)
        ).astype(bf)

    mask = np.triu(np.ones((128, 128), np.float32))
    # w1 [1024, 4096] -> [p, o4, ic, 512]; w2 [4096, 1024] -> [p, h4, hs, 1024]
    w1b = np.ascontiguousarray(
        w1.reshape(8, 128, 8, 512).transpose(1, 2, 0, 3)
    ).astype(bf)
    w2b = np.ascontiguousarray(
        w2.reshape(8, 4, 128, D).transpose(2, 0, 1, 3)
    ).astype(bf)
    # x[b].T -> [p, tq, ic, 512]
    xT_b = [
        np.ascontiguousarray(
            x[b].T.reshape(8, 128, 4, 512).transpose(1, 2, 0, 3)
        ).astype(bf)
        for b in range(B)
    ]
    in_maps = []
    for c in range(NCORES):
        b, q = c // 4, c % 4
        cols = slice(HCOLS * q, HCOLS * (q + 1))
        rows = slice(ROWS * q, ROWS * (q + 1))
        zm = np.zeros(NCORES, np.float32)
        zm[4 * b : 4 * b + 4] = 1.0
        in_maps.append(
            {
                "xbT": xT_b[b],
                "xr": np.ascontiguousarray(x[b, rows]),
                "wq_c": pmaj_in(np.ascontiguousarray(wq[:, cols]) * 0.125),
                "wk_c": pmaj_in(np.ascontiguousarray(wk[:, cols])),
                "wv_c": pmaj_in(np.ascontiguousarray(wv[:, cols])),
                "w1": w1b,
                "b1": b1,
                "w2": w2b,
                "mask_tri": mask,
                "zmask": zm,
            }
        )

    res = run_bass_kernel_spmd(nc, in_maps, list(range(NCORES)))
    outp = np.empty((B, L, D), np.float32)
    for c in range(NCORES):
        b, q = c // 4, c % 4
        outp[b, ROWS * q : ROWS * (q + 1)] = res.results[c]["out"]
    if getattr(res, "exec_time_ns", None) is not None:
        kernel.last_exec_time_ns = res.exec_time_ns
    return outp


kernel.last_exec_time_ns = None


# revision 15
# speedup vs baseline: 1.1709x; 1.1709x over previous
"""Self-contained Trainium2 Bass kernel for a post-LN transformer block.

Problem: y = LN(h + MLP(h)), h = LN(x + CausalAttn(x)), B=2, L=2048, D=1024,
H=16 heads, MLP hidden 4096, shared LN params, exact GELU, fp32 I/O.

Sharding (8 cores): core c handles batch b=c//4, head-group q=c%4 (heads
4q..4q+3) for attention, then rows [512q, 512q+512) of batch b for the
MLP/LN part. One 8-core AllToAll per half-row round re-shards from
column(head)-split to row-split (other-batch slots are zeroed via zmask so
receivers just add both batch halves). x arrives host-pre-transposed (xT).

Schedule: chunks are processed largest-first within each round (even round
2,4,6,0; odd round 7,5,3,1) so the serial exp tail of the LAST chunk is
minimal and the collective triggers as early as possible. K projection runs
in 256-token substeps just-in-time; V pairs are emitted inside the chunk
that first needs them. The whole normalize->zmask-mul->send-DMA->trigger
path is high-priority (zmask muls on the otherwise idle GpSimd engine) so
the Tile scheduler cannot delay a round's sends behind recv/LN work. MLP:
m1 in row-halves (half 0 only needs round A); m2 in tb-pairs with w2
half-resident in SBUF (h4 0-3 resident in slots freed after attention,
h4 4-7 streamed once per pair) so the two tbs of a pair finish staggered
and the final LN/DMA tail is short. Matmuls in bf16 with fp32 PSUM
accumulation; residuals/LN in fp32.
"""

import contextlib
import ctypes
import sys
import types

import numpy as np

B, L, D = 2, 2048, 1024
H, HD = 16, 64
DFF = 4 * D
EPS = 1e-5
NCORES = 8
ROWS = L // 4  # 512 rows per core for MLP phase
HPC = 4  # heads per core
HCOLS = HPC * HD  # 256 attn-out cols per core
NTB = L // 128  # 16 token blocks per batch
NRB = ROWS // 128  # 4 token blocks per core row-slice
NJ2 = L // 256  # 8 query chunks of 256


def _install_axon_hooks_shim():
    """Provide antenv.axon_hooks (NTFF profiling hook) when the image lacks it.

    Needed only when profiling (BASS_TRACE=1); harmless otherwise.
    """
    try:
        from antenv.axon_hooks import get_axon_ntff_profile_hook  # noqa: F401

        return
    except ImportError:
        pass
    try:
        import antenv
    except ImportError:
        return

    mod = types.ModuleType("antenv.axon_hooks")
    _state = {"hook": None}
    mod.set_axon_ntff_profile_hook = lambda h: _state.__setitem__("hook", h)
    mod.get_axon_ntff_profile_hook = lambda: _state["hook"]
    sys.modules["antenv.axon_hooks"] = mod
    antenv.axon_hooks = mod

    try:
        lib = ctypes.CDLL("/opt/axon/libaxon_pjrt.so")
    except OSError:
        return
    if not hasattr(lib, "axon_start_nrt_profile"):
        return
    lib.axon_start_nrt_profile.argtypes = [
        ctypes.POINTER(ctypes.c_int64),
        ctypes.c_size_t,
    ]
    lib.axon_start_nrt_profile.restype = ctypes.c_int64
    lib.axon_stop_nrt_profile.argtypes = [ctypes.c_char_p]
    lib.axon_stop_nrt_profile.restype = ctypes.c_int64

    @contextlib.contextmanager
    def _hook(output_dir, device_ids):
        import jax

        jax.devices()
        if device_ids:
            ids = (ctypes.c_int64 * len(device_ids))(*device_ids)
            rc = lib.axon_start_nrt_profile(ids, len(device_ids))
        else:
            rc = lib.axon_start_nrt_profile(None, 0)
        if rc != 0:
            raise RuntimeError(f"axon_start_nrt_profile rc={rc}")
        try:
            yield
        finally:
            n = lib.axon_stop_nrt_profile(str(output_dir).encode())
            print(f"profile: {n} file(s) -> {output_dir}", file=sys.stderr)

    mod.set_axon_ntff_profile_hook(_hook)


_install_axon_hooks_shim()

import concourse.bass as bass  # noqa: E402
import concourse.tile as tile  # noqa: E402
from concourse import bacc, mybir  # noqa: E402
from concourse.bass_utils import run_bass_kernel_spmd  # noqa: E402
from concourse.masks import make_identity  # noqa: E402

F32 = mybir.dt.float32
BF16 = mybir.dt.bfloat16

EVEN_ORDER = (2, 4, 6, 0)
ODD_ORDER = (7, 5, 3, 1)


def _build():
    nc = bacc.Bacc(
        "TRN2", target_bir_lowering=False, debug=False, num_devices=NCORES
    )

    def din(name, shape, dt=F32):
        return nc.dram_tensor(name, shape, dt, kind="ExternalInput").ap()

    # All large inputs are host-pre-arranged partition-major so every DMA
    # line is a long contiguous run (max descriptor efficiency).
    xbT = din("xbT", [128, 4, 8, 512], BF16)  # x[b].T as [p, tq, ic, tok]
    xr = din("xr", [ROWS, D], F32)  # this core's row slice of x, fp32
    wq_c = din("wq_c", [128, 8, HCOLS], BF16)  # [p, ic, col], pre-scaled 1/8
    wk_c = din("wk_c", [128, 8, HCOLS], BF16)
    wv_c = din("wv_c", [128, 8, HCOLS], BF16)
    w1 = din("w1", [128, 8, 8, 512], BF16)  # [p, o4, ic, col]
    b1 = din("b1", [DFF])
    w2 = din("w2", [128, 8, 4, D], BF16)  # [p, h4, hs, col]
    mask_tri = din("mask_tri", [128, 128])  # 1 where k<=q else 0
    zmask = din("zmask", [NCORES])  # 1 for same-batch a2a slots else 0
    out = nc.dram_tensor("out", [ROWS, D], F32, kind="ExternalOutput").ap()

    with tile.TileContext(nc) as tc, contextlib.ExitStack() as ctx:
        pb = ctx.enter_context(tc.tile_pool(name="pb", bufs=1))  # persistent
        pc = ctx.enter_context(tc.tile_pool(name="pc", bufs=1))  # constants
        pw = ctx.enter_context(tc.tile_pool(name="pw", bufs=1))  # resident W
        pws = ctx.enter_context(tc.tile_pool(name="pws", bufs=3))  # streamed W
        ps = ctx.enter_context(tc.tile_pool(name="ps", bufs=3))  # small tiles
        pr = ctx.enter_context(tc.tile_pool(name="pr", bufs=3))  # recv tiles
        pe = ctx.enter_context(tc.tile_pool(name="pe", bufs=4))  # exp tiles
        pp = ctx.enter_context(tc.tile_pool(name="pp", bufs=2, space="PSUM"))
        pd = ctx.enter_context(tc.tile_pool(name="pd", bufs=1, space="DRAM"))

        # ---- big SBUF tiles (tag-shared slots; lifetimes disjoint) ----
        xT = pb.tile([128, 4, 8, 512], BF16, tag="slotA")  # [p, tq, ic, tok]
        KT = pb.tile([128, 2, L], BF16, tag="slotC")  # dead after last scores
        QT = pb.tile([128, 2, L], BF16, tag="slotD")  # dead after last scores
        V_ext = pb.tile([128, NTB, HPC, HD + 1], BF16, tag="slotE")
        attn_sb = pb.tile([128, NTB, HCOLS], BF16, tag="slotF")
        res1 = pb.tile([128, NRB, D], F32, tag="slotG")
        hT = pb.tile([128, 8, ROWS], BF16, tag="slotH")

        # ---- startup DMAs, most-urgent first: wk + first xT half gate the
        #      first K substep; wq/wv go via the gpsimd queue in parallel ----
        # Startup loads split across the sync + gpsimd DMA queues (the
        # scalar/Activation queue is ~4x slower for bulk - only res1, not
        # needed until the first recv, goes there). Most-urgent first.
        wk_sb = pw.tile([128, 8, HCOLS], BF16)
        nc.sync.dma_start(out=wk_sb, in_=wk_c[:, :, :])
        nc.sync.dma_start(out=xT[:, 0, :, 0:256], in_=xbT[:, 0, :, 0:256])
        wq_sb = pw.tile([128, 8, HCOLS], BF16)
        nc.gpsimd.dma_start(out=wq_sb, in_=wq_c[:, :, :])
        wv_sb = pw.tile([128, 8, HCOLS], BF16)
        nc.gpsimd.dma_start(out=wv_sb, in_=wv_c[:, :, :])
        nc.sync.dma_start(out=xT[:, 0, :, 256:512], in_=xbT[:, 0, :, 256:512])
        nc.sync.dma_start(out=xT[:, 1, :, :], in_=xbT[:, 1, :, :])
        nc.gpsimd.dma_start(out=xT[:, 2, :, :], in_=xbT[:, 2, :, :])
        nc.sync.dma_start(out=xT[:, 3, :, :], in_=xbT[:, 3, :, :])

        # ---- early skew-absorbing barrier (tiny AllToAll; reads an
        #      uninitialized buffer so it has no upstream dependency).
        #      The first collective pays rank-launch skew + CC setup; this
        #      one absorbs it during compute so round A doesn't. ----
        bar_in = pd.tile([NCORES, 4], F32)
        bar_out = pd.tile([NCORES, 4], F32)
        nc.gpsimd.collective_compute(
            "AllToAll",
            mybir.AluOpType.bypass,
            replica_groups=[list(range(NCORES))],
            ins=[bar_in[:]],
            outs=[bar_out[:]],
        )

        # ---- constants ----
        ident_f = pc.tile([128, 128], F32)
        make_identity(nc, ident_f)
        ident_b = pc.tile([128, 128], BF16)
        make_identity(nc, ident_b)
        mask_sb = pc.tile([128, 128], BF16)
        nc.gpsimd.dma_start(out=mask_sb, in_=mask_tri[:, :])
        eps_sb = pc.tile([128, 1], F32)
        nc.vector.memset(eps_sb, EPS)
        b1_sb = pc.tile([128, 32], F32)  # per-partition bias for m1^T chunks
        nc.gpsimd.dma_start(
            out=b1_sb,
            in_=bass.AP(tensor=b1.tensor, offset=b1.offset, ap=[[1, 128], [128, 32]]),
        )
        zm_sb = pc.tile([128, NCORES], F32)
        nc.gpsimd.dma_start(
            out=zm_sb,
            in_=bass.AP(
                tensor=zmask.tensor, offset=zmask.offset, ap=[[0, 128], [1, NCORES]]
            ),
        )

        # ---- a2a DRAM buffers (bf16 payload, two half-row rounds; senders
        #      zero their payload toward other-batch receivers via zmask) ----
        a2a_in1 = pd.tile([NCORES, ROWS // 2, HCOLS], BF16)
        a2a_out1 = pd.tile([NCORES, ROWS // 2, HCOLS], BF16)
        a2a_in2 = pd.tile([NCORES, ROWS // 2, HCOLS], BF16)
        a2a_out2 = pd.tile([NCORES, ROWS // 2, HCOLS], BF16)

        # residual base for MLP rows arrives in the background
        nc.scalar.dma_start(out=res1, in_=xr.rearrange("(t p) c -> p t c", p=128))

        nc.vector.memset(V_ext[:, :, :, HD : HD + 1], 1.0)

        # ---- attention building blocks ----
        def q_slice(h, J2):
            p0 = 64 * (h % 2)
            return QT[p0 : p0 + 64, h // 2, J2 * 256 : (J2 + 1) * 256]

        def k_slice(h, kb):
            p0 = 64 * (h % 2)
            return KT[p0 : p0 + 64, h // 2, kb * 128 : (kb + 1) * 128]

        def k_sub(t):
            # K projection for 256-token substep t (tokens 256t..256t+256)
            psk = pp.tile([128, 2, 256], F32, tag="ps", name=f"psk_{t}")
            for oc in range(2):
                for ic in range(8):
                    nc.tensor.matmul(
                        psk[:, oc, :],
                        wk_sb[:, ic, oc * 128 : (oc + 1) * 128],
                        xT[:, t // 2, ic, (t % 2) * 256 : (t % 2) * 256 + 256],
                        start=(ic == 0),
                        stop=(ic == 7),
                    )
            nc.vector.tensor_copy(KT[:, :, t * 256 : (t + 1) * 256], psk)

        def q_proj(J2):
            tq, th = J2 // 2, (J2 % 2) * 256
            psq = pp.tile([128, 2, 256], F32, tag="pqv", name=f"psq_{J2}")
            for oc in range(2):
                for ic in range(8):
                    nc.tensor.matmul(
                        psq[:, oc, :],
                        wq_sb[:, ic, oc * 128 : (oc + 1) * 128],
                        xT[:, tq, ic, th : th + 256],
                        start=(ic == 0),
                        stop=(ic == 7),
                    )
            nc.vector.tensor_copy(QT[:, :, J2 * 256 : (J2 + 1) * 256], psq)

        def v_pair(tb2):
            psv = pp.tile([128, 2, 256], F32, tag="pqv", name=f"psv_{tb2}")
            for kk in range(2):
                tb = tb2 + kk
                for ic in range(8):
                    nc.tensor.matmul(
                        psv[:, kk, :],
                        xT[:, tb // 4, ic, (tb % 4) * 128 : (tb % 4) * 128 + 128],
                        wv_sb[:, ic, :],
                        start=(ic == 0),
                        stop=(ic == 7),
                    )
            nc.vector.tensor_copy(
                V_ext[:, tb2 : tb2 + 2, :, 0:HD],
                psv.rearrange("p k (h d) -> p k h d", h=HPC),
            )

        def process_chunk(J2, ain, hooks=None):
            """Scores -> exp -> AV -> normalize -> a2a sends for one 256-query
            chunk. hooks[kp] emits prerequisite K substeps / V pairs."""
            for hp in range(2):
                h0, h1 = 2 * hp, 2 * hp + 1
                psu = pp.tile(
                    [128, 2, 2, HD + 1], F32, tag="pu", name=f"psu_{J2}_{hp}"
                )
                exps = [None] * (J2 + 1)

                def av_quad(kp, J2=J2, hp=hp, psu=psu, exps=exps):
                    # psu packs 4 accumulation regions (hh, js) in ONE psum
                    # bank. start=True marks the WHOLE bank pending-zero, so
                    # only the very first matmul into the bank may carry it:
                    # each region's first write then consumes its pending
                    # bytes (overwrite), later writes accumulate.
                    expP = exps[kp]
                    for idx in range(4):
                        hh = idx // 2  # 0 -> h0, 1 -> h1
                        kb = 2 * kp + (idx % 2)
                        hg = 2 * hp + hh
                        for js in range(2):
                            if 2 * J2 + js < kb:
                                continue
                            nc.tensor.matmul(
                                psu[:, hh, js, :],
                                expP[:, idx, js * 128 : (js + 1) * 128],
                                V_ext[:, kb, hg, :],
                                start=(kb == 0 and idx == 0 and js == 0),
                                stop=(kb == 2 * J2 + js),
                            )

                for kp in range(J2 + 1):
                    if hp == 0 and hooks and kp in hooks:
                        for fn in hooks[kp]:
                            fn()
                    k0, k1 = 2 * kp, 2 * kp + 1
                    pssP = pp.tile(
                        [128, 4, 256], F32, tag="ps", name=f"pssP_{J2}_{hp}_{kp}"
                    )
                    # bank0 <- head h0 (rows 0-63), bank1 <- head h1 (rows
                    # 64-127); pairs target disjoint row groups + banks so
                    # they run concurrently in the PE array.
                    nc.tensor.matmul(
                        pssP[:, 0, :], k_slice(h0, k0), q_slice(h0, J2),
                        start=True, stop=True,
                    )
                    nc.tensor.matmul(
                        pssP[:, 2, :], k_slice(h1, k0), q_slice(h1, J2),
                        start=True, stop=True,
                    )
                    nc.tensor.matmul(
                        pssP[:, 1, :], k_slice(h0, k1), q_slice(h0, J2),
                        start=True, stop=True,
                    )
                    nc.tensor.matmul(
                        pssP[:, 3, :], k_slice(h1, k1), q_slice(h1, J2),
                        start=True, stop=True,
                    )
                    expP = pe.tile([128, 4, 256], BF16, tag="expT",
                                   name=f"expP_{J2}_{hp}_{kp}")
                    nc.scalar.activation(
                        expP, pssP, mybir.ActivationFunctionType.Exp
                    )
                    if kp == J2:  # diagonal pair: causal mask inside
                        for idx, js in ((0, 0), (1, 1), (2, 0), (3, 1)):
                            nc.vector.tensor_mul(
                                expP[:, idx, js * 128 : (js + 1) * 128],
                                expP[:, idx, js * 128 : (js + 1) * 128],
                                mask_sb,
                            )
                    exps[kp] = expP
                    # 2-unit lookahead: av_quad(kp-2) consumes an exp that
                    # has had two scores-units (~1.1us) of PE time to finish,
                    # so the PE never stalls on the ~0.85us exp latency.
                    if kp >= 2:
                        av_quad(kp - 2)
                if J2 >= 1:
                    av_quad(J2 - 1)
                av_quad(J2)
                # softmax normalize + write attn_sb columns for this pair.
                # High priority: the sends (and so the collective trigger)
                # depend on these; don't let the scheduler defer them.
                with tc.high_priority():
                    for hh in range(2):
                        hg = 2 * hp + hh
                        for js in range(2):
                            rec = ps.tile([128, 1], F32, tag="rec")
                            nc.vector.reciprocal(
                                rec, psu[:, hh, js, HD : HD + 1]
                            )
                            nc.vector.tensor_scalar_mul(
                                attn_sb[:, 2 * J2 + js, hg * HD : (hg + 1) * HD],
                                psu[:, hh, js, 0:HD],
                                rec,
                            )
            # ship this chunk's two token blocks to both batch slots (the
            # other-batch copy is zeroed so receivers just add both). All
            # high priority so the round's trigger fires as soon as possible
            # (gpsimd is unusable here: its TENSOR_SCALAR is ~8us/op).
            with tc.high_priority():
                for s in (J2 // 2, 4 + J2 // 2):
                    st = pr.tile(
                        [128, 2, HCOLS], BF16, tag="st", name=f"st_{J2}_{s}"
                    )
                    nc.vector.tensor_scalar_mul(
                        st, attn_sb[:, 2 * J2 : 2 * J2 + 2, :], zm_sb[:, s : s + 1]
                    )
                    nc.sync.dma_start(
                        out=ain[s].rearrange("(t p) c -> p t c", p=128), in_=st
                    )

        # ---- attention: biggest chunk of each round first so the round's
        #      last (smallest) chunk has a tiny exp tail and the collective
        #      triggers right after the round's PE work ends ----
        k_sub(0)
        q_proj(2)
        v_pair(0)
        process_chunk(2, a2a_in1, hooks={
            1: [lambda: k_sub(1), lambda: v_pair(2)],
            2: [lambda: k_sub(2), lambda: v_pair(4)],
        })
        q_proj(4)
        process_chunk(4, a2a_in1, hooks={
            3: [lambda: k_sub(3), lambda: v_pair(6)],
            4: [lambda: k_sub(4), lambda: v_pair(8)],
        })
        q_proj(6)
        process_chunk(6, a2a_in1, hooks={
            5: [lambda: k_sub(5), lambda: v_pair(10)],
            6: [lambda: k_sub(6), lambda: v_pair(12)],
        })
        q_proj(0)
        process_chunk(0, a2a_in1)
        with tc.high_priority():
            nc.gpsimd.collective_compute(
                "AllToAll",
                mybir.AluOpType.bypass,
                replica_groups=[list(range(NCORES))],
                ins=[a2a_in1[:]],
                outs=[a2a_out1[:]],
            )

        k_sub(7)
        v_pair(14)
        q_proj(7)
        process_chunk(7, a2a_in2)
        q_proj(5)
        process_chunk(5, a2a_in2)
        q_proj(3)
        process_chunk(3, a2a_in2)
        q_proj(1)
        process_chunk(1, a2a_in2)
        with tc.high_priority():
            nc.gpsimd.collective_compute(
                "AllToAll",
                mybir.AluOpType.bypass,
                replica_groups=[list(range(NCORES))],
                ins=[a2a_in2[:]],
                outs=[a2a_out2[:]],
            )

        # ---- resident slice of w2 (h4 0-1) into the SBUF slots that die
        #      with the attention phase (exact-size fits, no slot growth);
        #      h4 2-7 are streamed per m2 pair ----
        w2resA = pb.tile([128, 4, D], BF16, tag="slotE")
        nc.sync.dma_start(out=w2resA, in_=w2[:, 0, :, :])
        w2resB = pb.tile([128, 4, D], BF16, tag="slotF")
        nc.sync.dma_start(out=w2resB, in_=w2[:, 1, :, :])

        # ---- recv + LN1 + transpose to hT, then m1 in token halves so the
        #      round-A half starts while round B's collective drains ----
        h_sb = pb.tile([128, NRB, D], F32, tag="slotD")  # reuses QT slot
        h_bf = pb.tile([128, NRB, D], BF16, tag="slotI")  # bf16 copy for hT
        res2 = pb.tile([128, NRB, D], F32, tag="slotC")  # reuses KT slot
        gT = pb.tile([128, 32, ROWS], BF16, tag="slotA")  # reuses xT slot

        def recv_adds(tb, aout, ti):
            # sync-issued DMAs (collective-completion deps enforced there),
            # emitted only after all a2a sends so those never block; adds
            # split gpsimd/vector by column group (disjoint res1 ranges) so
            # each token block's chain runs on two engines concurrently and
            # the vector share stays small (it also carries LN1 + the odd
            # round's normalize in this window)
            for g in range(4):
                eng = nc.gpsimd if g < 2 else nc.vector
                r0 = pr.tile([128, HCOLS], BF16, tag="r0", name=f"r0_{tb}_{g}")
                nc.sync.dma_start(
                    out=r0,
                    in_=aout[g].rearrange("(t p) c -> p t c", p=128)[:, ti, :],
                )
                r1 = pr.tile([128, HCOLS], BF16, tag="r1", name=f"r1_{tb}_{g}")
                nc.sync.dma_start(
                    out=r1,
                    in_=aout[4 + g].rearrange("(t p) c -> p t c", p=128)[
                        :, ti, :
                    ],
                )
                # exactly one of the pair is nonzero (zmask), so the bf16
                # intermediate sum is exact
                ta = pr.tile([128, HCOLS], BF16, tag="ta", name=f"ta_{tb}_{g}")
                eng.tensor_add(ta, r0, r1)
                dst = res1[:, tb, g * HCOLS : (g + 1) * HCOLS]
                eng.tensor_add(dst, dst, ta)

        def ln_row(src_t, tb, out_ap, bf_ap=None):
            stats = ps.tile([128, 2, 6], F32, tag="stats")
            nc.vector.bn_stats(stats[:, 0, :], src_t[:, tb, 0:512])
            nc.vector.bn_stats(stats[:, 1, :], src_t[:, tb, 512:1024])
            mv = ps.tile([128, 2], F32, tag="mv")
            nc.vector.bn_aggr(mv, stats)
            std = ps.tile([128, 1], F32, tag="std")
            nc.scalar.activation(
                std, mv[:, 1:2], mybir.ActivationFunctionType.Sqrt,
                bias=eps_sb[:, 0:1], scale=1.0,
            )
            rstd = ps.tile([128, 1], F32, tag="rstd")
            nc.vector.reciprocal(rstd, std)
            # ln_g == 1, ln_b == 0 in this problem, so affine is identity
            nc.vector.tensor_scalar(
                out=out_ap,
                in0=src_t[:, tb, :],
                scalar1=mv[:, 0:1],
                scalar2=rstd,
                op0=mybir.AluOpType.subtract,
                op1=mybir.AluOpType.mult,
            )
            if bf_ap is not None:
                # bf16 shadow copy on the (idle-here) scalar engine
                nc.scalar.copy(bf_ap, out_ap)

        def ln_hT(tb):
            ln_row(res1, tb, h_sb[:, tb, :], h_bf[:, tb, :])
            for f4 in range(2):
                psT = pp.tile(
                    [128, 4, 128], BF16, tag="pu", name=f"psT_{tb}_{f4}"
                )
                for fs in range(4):
                    fc = 4 * f4 + fs
                    nc.tensor.transpose(
                        psT[:, fs, :],
                        h_bf[:, tb, fc * 128 : (fc + 1) * 128],
                        ident_b,
                    )
                nc.vector.tensor_copy(
                    hT[:, 4 * f4 : 4 * f4 + 4, tb * 128 : (tb + 1) * 128],
                    psT,
                )

        def m1_half(half):
            c0 = 256 * half
            for o4 in range(8):
                w1c = pws.tile(
                    [128, 8, 512], BF16, tag="w1c", name=f"w1c_{half}_{o4}"
                )
                nc.sync.dma_start(out=w1c, in_=w1[:, o4, :, :])
                for os_ in range(4):
                    oc = o4 * 4 + os_
                    psm = pp.tile([128, 256], F32, tag="pqv", name=f"psm_{half}_{oc}")
                    for ic in range(8):
                        nc.tensor.matmul(
                            psm,
                            w1c[:, ic, os_ * 128 : (os_ + 1) * 128],
                            hT[:, ic, c0 : c0 + 256],
                            start=(ic == 0),
                            stop=(ic == 7),
                        )
                    nc.scalar.activation(
                        gT[:, oc, c0 : c0 + 256], psm,
                        mybir.ActivationFunctionType.Gelu,
                        bias=b1_sb[:, oc : oc + 1], scale=1.0,
                    )

        def m2_pair(tbp):
            tbs = (2 * tbp, 2 * tbp + 1)
            pso = {
                tb: pp.tile([128, 2, 512], F32, tag="ps", name=f"pso_{tb}")
                for tb in tbs
            }
            # streamed phase: h4 2..7, each w2c shared by both tbs
            for h4 in range(2, 8):
                w2c = pws.tile(
                    [128, 4, D], BF16, tag="w2c", name=f"w2c_{tbp}_{h4}"
                )
                nc.sync.dma_start(out=w2c, in_=w2[:, h4, :, :])
                for hs in range(4):
                    for tb in tbs:
                        for f2 in range(2):
                            nc.tensor.matmul(
                                pso[tb][:, f2, :],
                                gT[:, 4 * h4 + hs, tb * 128 : (tb + 1) * 128],
                                w2c[:, hs, f2 * 512 : (f2 + 1) * 512],
                                start=(h4 == 2 and hs == 0),
                                stop=False,
                            )
            # resident phase per tb (h4 0..1), staggered so the first tb's
            # evacuate/LN/DMA overlaps the second tb's matmuls
            for tb in tbs:
                for h4 in range(2):
                    wr = w2resA if h4 == 0 else w2resB
                    for hs in range(4):
                        for f2 in range(2):
                            nc.tensor.matmul(
                                pso[tb][:, f2, :],
                                gT[:, 4 * h4 + hs, tb * 128 : (tb + 1) * 128],
                                wr[:, hs, f2 * 512 : (f2 + 1) * 512],
                                start=False,
                                stop=(h4 == 1 and hs == 3),
                            )
                # b2 == 0 in this problem (skipped)
                nc.vector.tensor_add(
                    res2[:, tb, :],
                    pso[tb].rearrange("p a b -> p (a b)"),
                    h_sb[:, tb, :],
                )
                o_t = ps.tile([128, D], F32, tag="o_t", bufs=2)
                ln_row(res2, tb, o_t)
                nc.sync.dma_start(out=out[tb * 128 : (tb + 1) * 128, :], in_=o_t)

        # half 0 (round A rows) first; round B's recv + LN overlap m1/m2
        # of half 0, so the PE never waits on the second collective.
        recv_adds(0, a2a_out1, 0)
        recv_adds(1, a2a_out1, 1)
        ln_hT(0)
        ln_hT(1)
        m1_half(0)
        recv_adds(2, a2a_out2, 0)
        recv_adds(3, a2a_out2, 1)
        ln_hT(2)
        ln_hT(3)
        m2_pair(0)
        m1_half(1)
        m2_pair(1)

    nc.compile()
    return nc


_NC_CACHE = [None]


def kernel(**inputs) -> np.ndarray:
    import ml_dtypes

    x = np.asarray(inputs["x"], np.float32)
    wq = np.asarray(inputs["wq"], np.float32)
    wk = np.asarray(inputs["wk"], np.float32)
    wv = np.asarray(inputs["wv"], np.float32)
    w1 = np.asarray(inputs["w1"], np.float32)
    b1 = np.asarray(inputs["b1"], np.float32)
    w2 = np.asarray(inputs["w2"], np.float32)

    # The kernel folds these away; setup_inputs() constructs them as
    # zeros/ones. Fail loudly if that ever changes.
    for nm in ("bq", "bk", "bv", "b2"):
        if nm in inputs:
            assert not np.any(np.asarray(inputs[nm])), f"{nm} expected zero"
    if "ln_b" in inputs:
        assert not np.any(np.asarray(inputs["ln_b"])), "ln_b expected zero"
    if "ln_g" in inputs:
        assert np.all(np.asarray(inputs["ln_g"]) == 1.0), "ln_g expected ones"

    if _NC_CACHE[0] is None:
        _NC_CACHE[0] = _build()
    nc = _NC_CACHE[0]

    bf = ml_dtypes.bfloat16

    def pmaj_in(m):  # [D, cols] -> [p, ic, cols] partition-major
        return np.ascontiguousarray(
            m.reshape(8, 128, m.shape[1]).transpose(1, 0, 2)
        ).astype(bf)

    mask = np.triu(np.ones((128, 128), np.float32))
    # w1 [1024, 4096] -> [p, o4, ic, 512]; w2 [4096, 1024] -> [p, h4, hs, 1024]
    w1b = np.ascontiguousarray(
        w1.reshape(8, 128, 8, 512).transpose(1, 2, 0, 3)
    ).astype(bf)
    w2b = np.ascontiguousarray(
        w2.reshape(8, 4, 128, D).transpose(2, 0, 1, 3)
    ).astype(bf)
    # x[b].T -> [p, tq, ic, 512]
    xT_b = [
        np.ascontiguousarray(
            x[b].T.reshape(8, 128, 4, 512).transpose(1, 2, 0, 3)
        ).astype(bf)
        for b in range(B)
    ]
    in_maps = []
    for c in range(NCORES):
        b, q = c // 4, c % 4
        cols = slice(HCOLS * q, HCOLS * (q + 1))
        rows = slice(ROWS * q, ROWS * (q + 1))
        zm = np.zeros(NCORES, np.float32)
        zm[4 * b : 4 * b + 4] = 1.0
        in_maps.append(
            {
                "xbT": xT_b[b],
                "xr": np.ascontiguousarray(x[b, rows]),
                "wq_c": pmaj_in(np.ascontiguousarray(wq[:, cols]) * 0.125),
                "wk_c": pmaj_in(np.ascontiguousarray(wk[:, cols])),
                "wv_c": pmaj_in(np.ascontiguousarray(wv[:, cols])),
                "w1": w1b,
                "b1": b1,
                "w2": w2b,
                "mask_tri": mask,
                "zmask": zm,
            }
        )

    res = run_bass_kernel_spmd(nc, in_maps, list(range(NCORES)))
    outp = np.empty((B, L, D), np.float32)
    for c in range(NCORES):
        b, q = c // 4, c % 4
        outp[b, ROWS * q : ROWS * (q + 1)] = res.results[c]["out"]
    if getattr(res, "exec_time_ns", None) is not None:
        kernel.last_exec_time_ns = res.exec_time_ns
    return outp


kernel.last_exec_time_ns = None


# revision 24
# speedup vs baseline: 1.1812x; 1.0088x over previous
"""Self-contained Trainium2 Bass kernel for a post-LN transformer block.

Problem: y = LN(h + MLP(h)), h = LN(x + CausalAttn(x)), B=2, L=2048, D=1024,
H=16 heads, MLP hidden 4096, shared LN params, exact GELU, fp32 I/O.

Sharding (8 cores): core c handles batch b=c//4, head-group q=c%4 (heads
4q..4q+3) for attention, then rows [512q, 512q+512) of batch b for the
MLP/LN part. One 8-core AllToAll per half-row round re-shards from
column(head)-split to row-split (other-batch slots are zeroed via zmask so
receivers just add both batch halves). x arrives host-pre-transposed (xT).

Schedule: chunks are processed largest-first within each round (even round
2,4,6,0; odd round 7,5,3,1) so the serial exp tail of the LAST chunk is
minimal and the collective triggers as early as possible. K projection runs
in 256-token substeps just-in-time; V pairs are emitted inside the chunk
that first needs them. The whole normalize->zmask-mul->send-DMA->trigger
path is high-priority (zmask muls on the otherwise idle GpSimd engine) so
the Tile scheduler cannot delay a round's sends behind recv/LN work. MLP:
m1 in row-halves (half 0 only needs round A); m2 in tb-pairs with w2
half-resident in SBUF (h4 0-3 resident in slots freed after attention,
h4 4-7 streamed once per pair) so the two tbs of a pair finish staggered
and the final LN/DMA tail is short. Matmuls in bf16 with fp32 PSUM
accumulation; residuals/LN in fp32.
"""

import contextlib
import ctypes
import sys
import types

import numpy as np

B, L, D = 2, 2048, 1024
H, HD = 16, 64
DFF = 4 * D
EPS = 1e-5
NCORES = 8
ROWS = L // 4  # 512 rows per core for MLP phase
HPC = 4  # heads per core
HCOLS = HPC * HD  # 256 attn-out cols per core
NTB = L // 128  # 16 token blocks per batch
NRB = ROWS // 128  # 4 token blocks per core row-slice
NJ2 = L // 256  # 8 query chunks of 256


def _install_axon_hooks_shim():
    """Provide antenv.axon_hooks (NTFF profiling hook) when the image lacks it.

    Needed only when profiling (BASS_TRACE=1); harmless otherwise.
    """
    try:
        from antenv.axon_hooks import get_axon_ntff_profile_hook  # noqa: F401

        return
    except ImportError:
        pass
    try:
        import antenv
    except ImportError:
        return

    mod = types.ModuleType("antenv.axon_hooks")
    _state = {"hook": None}
    mod.set_axon_ntff_profile_hook = lambda h: _state.__setitem__("hook", h)
    mod.get_axon_ntff_profile_hook = lambda: _state["hook"]
    sys.modules["antenv.axon_hooks"] = mod
    antenv.axon_hooks = mod

    try:
        lib = ctypes.CDLL("/opt/axon/libaxon_pjrt.so")
    except OSError:
        return
    if not hasattr(lib, "axon_start_nrt_profile"):
        return
    lib.axon_start_nrt_profile.argtypes = [
        ctypes.POINTER(ctypes.c_int64),
        ctypes.c_size_t,
    ]
    lib.axon_start_nrt_profile.restype = ctypes.c_int64
    lib.axon_stop_nrt_profile.argtypes = [ctypes.c_char_p]
    lib.axon_stop_nrt_profile.restype = ctypes.c_int64

    @contextlib.contextmanager
    def _hook(output_dir, device_ids):
        import jax

        jax.devices()
        if device_ids:
            ids = (ctypes.c_int64 * len(device_ids))(*device_ids)
            rc = lib.axon_start_nrt_profile(ids, len(device_ids))
        else:
            rc = lib.axon_start_nrt_profile(None, 0)
        if rc != 0:
            raise RuntimeError(f"axon_start_nrt_profile rc={rc}")
        try:
            yield
        finally:
            n = lib.axon_stop_nrt_profile(str(output_dir).encode())
            print(f"profile: {n} file(s) -> {output_dir}", file=sys.stderr)

    mod.set_axon_ntff_profile_hook(_hook)


_install_axon_hooks_shim()

import concourse.bass as bass  # noqa: E402
import concourse.tile as tile  # noqa: E402
from concourse import bacc, mybir  # noqa: E402
from concourse.bass_utils import run_bass_kernel_spmd  # noqa: E402
from concourse.masks import make_identity  # noqa: E402

F32 = mybir.dt.float32
BF16 = mybir.dt.bfloat16

EVEN_ORDER = (2, 4, 6, 0)
ODD_ORDER = (7, 5, 3, 1)


def _build():
    nc = bacc.Bacc(
        "TRN2", target_bir_lowering=False, debug=False, num_devices=NCORES
    )

    def din(name, shape, dt=F32):
        return nc.dram_tensor(name, shape, dt, kind="ExternalInput").ap()

    # All large inputs are host-pre-arranged partition-major so every DMA
    # line is a long contiguous run (max descriptor efficiency).
    xbT = din("xbT", [128, 4, 8, 512], BF16)  # x[b].T as [p, tq, ic, tok]
    xr = din("xr", [ROWS, D], F32)  # this core's row slice of x, fp32
    wq_c = din("wq_c", [128, 8, HCOLS], BF16)  # [p, ic, col], pre-scaled 1/8
    wk_c = din("wk_c", [128, 8, HCOLS], BF16)
    wv_c = din("wv_c", [128, 8, HCOLS], BF16)
    w1 = din("w1", [128, 8, 8, 512], BF16)  # [p, o4, ic, col]
    b1 = din("b1", [DFF])
    w2 = din("w2", [128, 8, 4, D], BF16)  # [p, h4, hs, col]
    mask_tri = din("mask_tri", [128, 128])  # 1 where k<=q else 0
    zmask = din("zmask", [NCORES])  # 1 for same-batch a2a slots else 0
    out = nc.dram_tensor("out", [ROWS, D], F32, kind="ExternalOutput").ap()

    with tile.TileContext(nc) as tc, contextlib.ExitStack() as ctx:
        pb = ctx.enter_context(tc.tile_pool(name="pb", bufs=1))  # persistent
        pc = ctx.enter_context(tc.tile_pool(name="pc", bufs=1))  # constants
        pw = ctx.enter_context(tc.tile_pool(name="pw", bufs=1))  # resident W
        pws = ctx.enter_context(tc.tile_pool(name="pws", bufs=3))  # streamed W
        ps = ctx.enter_context(tc.tile_pool(name="ps", bufs=3))  # small tiles
        pr = ctx.enter_context(tc.tile_pool(name="pr", bufs=3))  # recv tiles
        pe = ctx.enter_context(tc.tile_pool(name="pe", bufs=4))  # exp tiles
        pp = ctx.enter_context(tc.tile_pool(name="pp", bufs=2, space="PSUM"))
        pd = ctx.enter_context(tc.tile_pool(name="pd", bufs=1, space="DRAM"))

        # ---- big SBUF tiles (tag-shared slots; lifetimes disjoint) ----
        xT = pb.tile([128, 4, 8, 512], BF16, tag="slotA")  # [p, tq, ic, tok]
        KT = pb.tile([128, 2, L], BF16, tag="slotC")  # dead after last scores
        QT = pb.tile([128, 2, L], BF16, tag="slotD")  # dead after last scores
        V_ext = pb.tile([128, NTB, HPC, HD + 1], BF16, tag="slotE")
        attn_sb = pb.tile([128, NTB, HCOLS], BF16, tag="slotF")
        res1 = pb.tile([128, NRB, D], F32, tag="slotG")
        hT = pb.tile([128, 8, ROWS], BF16, tag="slotH")

        # ---- startup DMAs, most-urgent first: wk + first xT half gate the
        #      first K substep; wq/wv go via the gpsimd queue in parallel ----
        # Startup loads split across the sync + gpsimd DMA queues (the
        # scalar/Activation queue is ~4x slower for bulk - only res1, not
        # needed until the first recv, goes there). Most-urgent first.
        wk_sb = pw.tile([128, 8, HCOLS], BF16)
        nc.sync.dma_start(out=wk_sb, in_=wk_c[:, :, :])
        nc.sync.dma_start(out=xT[:, 0, :, 0:256], in_=xbT[:, 0, :, 0:256])
        wq_sb = pw.tile([128, 8, HCOLS], BF16)
        nc.gpsimd.dma_start(out=wq_sb, in_=wq_c[:, :, :])
        wv_sb = pw.tile([128, 8, HCOLS], BF16)
        nc.gpsimd.dma_start(out=wv_sb, in_=wv_c[:, :, :])
        nc.sync.dma_start(out=xT[:, 0, :, 256:512], in_=xbT[:, 0, :, 256:512])
        nc.gpsimd.dma_start(out=xT[:, 1, :, :], in_=xbT[:, 1, :, :])
        nc.sync.dma_start(out=xT[:, 2, :, :], in_=xbT[:, 2, :, :])
        nc.sync.dma_start(out=xT[:, 3, :, :], in_=xbT[:, 3, :, :])

        # ---- early skew-absorbing barrier (tiny AllToAll; reads an
        #      uninitialized buffer so it has no upstream dependency).
        #      The first collective pays rank-launch skew + CC setup; this
        #      one absorbs it during compute so round A doesn't. ----
        bar_in = pd.tile([NCORES, 4], F32)
        bar_out = pd.tile([NCORES, 4], F32)
        nc.gpsimd.collective_compute(
            "AllToAll",
            mybir.AluOpType.bypass,
            replica_groups=[list(range(NCORES))],
            ins=[bar_in[:]],
            outs=[bar_out[:]],
        )

        # ---- constants ----
        ident_f = pc.tile([128, 128], F32)
        make_identity(nc, ident_f)
        ident_b = pc.tile([128, 128], BF16)
        make_identity(nc, ident_b)
        mask_sb = pc.tile([128, 128], BF16)
        nc.gpsimd.dma_start(out=mask_sb, in_=mask_tri[:, :])
        eps_sb = pc.tile([128, 1], F32)
        nc.vector.memset(eps_sb, EPS)
        b1_sb = pc.tile([128, 32], F32)  # per-partition bias for m1^T chunks
        nc.gpsimd.dma_start(
            out=b1_sb,
            in_=bass.AP(tensor=b1.tensor, offset=b1.offset, ap=[[1, 128], [128, 32]]),
        )
        zm_sb = pc.tile([128, NCORES], F32)
        nc.gpsimd.dma_start(
            out=zm_sb,
            in_=bass.AP(
                tensor=zmask.tensor, offset=zmask.offset, ap=[[0, 128], [1, NCORES]]
            ),
        )

        # ---- a2a DRAM buffers (bf16 payload, two half-row rounds; senders
        #      zero their payload toward other-batch receivers via zmask) ----
        a2a_in1 = pd.tile([NCORES, ROWS // 2, HCOLS], BF16)
        a2a_out1 = pd.tile([NCORES, ROWS // 2, HCOLS], BF16)
        a2a_in2 = pd.tile([NCORES, ROWS // 2, HCOLS], BF16)
        a2a_out2 = pd.tile([NCORES, ROWS // 2, HCOLS], BF16)

        # residual base for MLP rows arrives in the background
        nc.scalar.dma_start(out=res1, in_=xr.rearrange("(t p) c -> p t c", p=128))

        nc.vector.memset(V_ext[:, :, :, HD : HD + 1], 1.0)

        # ---- attention building blocks ----
        def q_slice(h, J2):
            p0 = 64 * (h % 2)
            return QT[p0 : p0 + 64, h // 2, J2 * 256 : (J2 + 1) * 256]

        def k_slice(h, kb):
            p0 = 64 * (h % 2)
            return KT[p0 : p0 + 64, h // 2, kb * 128 : (kb + 1) * 128]

        def k_sub(t):
            # K projection for 256-token substep t (tokens 256t..256t+256)
            psk = pp.tile([128, 2, 256], F32, tag="ps", name=f"psk_{t}")
            for oc in range(2):
                for ic in range(8):
                    nc.tensor.matmul(
                        psk[:, oc, :],
                        wk_sb[:, ic, oc * 128 : (oc + 1) * 128],
                        xT[:, t // 2, ic, (t % 2) * 256 : (t % 2) * 256 + 256],
                        start=(ic == 0),
                        stop=(ic == 7),
                    )
            nc.vector.tensor_copy(KT[:, :, t * 256 : (t + 1) * 256], psk)

        def q_proj(J2):
            tq, th = J2 // 2, (J2 % 2) * 256
            psq = pp.tile([128, 2, 256], F32, tag="pqv", name=f"psq_{J2}")
            for oc in range(2):
                for ic in range(8):
                    nc.tensor.matmul(
                        psq[:, oc, :],
                        wq_sb[:, ic, oc * 128 : (oc + 1) * 128],
                        xT[:, tq, ic, th : th + 256],
                        start=(ic == 0),
                        stop=(ic == 7),
                    )
            nc.vector.tensor_copy(QT[:, :, J2 * 256 : (J2 + 1) * 256], psq)

        def v_pair(tb2):
            psv = pp.tile([128, 2, 256], F32, tag="pqv", name=f"psv_{tb2}")
            for kk in range(2):
                tb = tb2 + kk
                for ic in range(8):
                    nc.tensor.matmul(
                        psv[:, kk, :],
                        xT[:, tb // 4, ic, (tb % 4) * 128 : (tb % 4) * 128 + 128],
                        wv_sb[:, ic, :],
                        start=(ic == 0),
                        stop=(ic == 7),
                    )
            nc.vector.tensor_copy(
                V_ext[:, tb2 : tb2 + 2, :, 0:HD],
                psv.rearrange("p k (h d) -> p k h d", h=HPC),
            )

        def process_chunk(J2, ain, hooks=None):
            """Scores -> exp -> AV -> normalize -> a2a sends for one 256-query
            chunk. hooks[kp] emits prerequisite K substeps / V pairs."""
            for hp in range(2):
                h0, h1 = 2 * hp, 2 * hp + 1
                psu = pp.tile(
                    [128, 2, 2, HD + 1], F32, tag="pu", name=f"psu_{J2}_{hp}"
                )
                exps = [None] * (J2 + 1)

                def av_quad(kp, J2=J2, hp=hp, psu=psu, exps=exps):
                    # psu packs 4 accumulation regions (hh, js) in ONE psum
                    # bank. start=True marks the WHOLE bank pending-zero, so
                    # only the very first matmul into the bank may carry it:
                    # each region's first write then consumes its pending
                    # bytes (overwrite), later writes accumulate.
                    expP = exps[kp]
                    for idx in range(4):
                        hh = idx // 2  # 0 -> h0, 1 -> h1
                        kb = 2 * kp + (idx % 2)
                        hg = 2 * hp + hh
                        for js in range(2):
                            if 2 * J2 + js < kb:
                                continue
                            nc.tensor.matmul(
                                psu[:, hh, js, :],
                                expP[:, idx, js * 128 : (js + 1) * 128],
                                V_ext[:, kb, hg, :],
                                start=(kb == 0 and idx == 0 and js == 0),
                                stop=(kb == 2 * J2 + js),
                            )

                for kp in range(J2 + 1):
                    if hp == 0 and hooks and kp in hooks:
                        for fn in hooks[kp]:
                            fn()
                    k0, k1 = 2 * kp, 2 * kp + 1
                    pssP = pp.tile(
                        [128, 4, 256], F32, tag="ps", name=f"pssP_{J2}_{hp}_{kp}"
                    )
                    # bank0 <- head h0 (rows 0-63), bank1 <- head h1 (rows
                    # 64-127); pairs target disjoint row groups + banks so
                    # they run concurrently in the PE array.
                    nc.tensor.matmul(
                        pssP[:, 0, :], k_slice(h0, k0), q_slice(h0, J2),
                        start=True, stop=True,
                    )
                    nc.tensor.matmul(
                        pssP[:, 2, :], k_slice(h1, k0), q_slice(h1, J2),
                        start=True, stop=True,
                    )
                    nc.tensor.matmul(
                        pssP[:, 1, :], k_slice(h0, k1), q_slice(h0, J2),
                        start=True, stop=True,
                    )
                    nc.tensor.matmul(
                        pssP[:, 3, :], k_slice(h1, k1), q_slice(h1, J2),
                        start=True, stop=True,
                    )
                    expP = pe.tile([128, 4, 256], BF16, tag="expT",
                                   name=f"expP_{J2}_{hp}_{kp}")
                    nc.scalar.activation(
                        expP, pssP, mybir.ActivationFunctionType.Exp
                    )
                    if kp == J2:  # diagonal pair: causal mask inside
                        for idx, js in ((0, 0), (1, 1), (2, 0), (3, 1)):
                            nc.vector.tensor_mul(
                                expP[:, idx, js * 128 : (js + 1) * 128],
                                expP[:, idx, js * 128 : (js + 1) * 128],
                                mask_sb,
                            )
                    exps[kp] = expP
                    # 2-unit lookahead: av_quad(kp-2) consumes an exp that
                    # has had two scores-units (~1.1us) of PE time to finish,
                    # so the PE never stalls on the ~0.85us exp latency.
                    if kp >= 2:
                        av_quad(kp - 2)
                if J2 >= 1:
                    av_quad(J2 - 1)
                av_quad(J2)
                # softmax normalize + write attn_sb columns for this pair.
                # High priority: the sends (and so the collective trigger)
                # depend on these; don't let the scheduler defer them.
                with tc.high_priority():
                    for hh in range(2):
                        hg = 2 * hp + hh
                        for js in range(2):
                            rec = ps.tile([128, 1], F32, tag="rec")
                            nc.vector.reciprocal(
                                rec, psu[:, hh, js, HD : HD + 1]
                            )
                            nc.vector.tensor_scalar_mul(
                                attn_sb[:, 2 * J2 + js, hg * HD : (hg + 1) * HD],
                                psu[:, hh, js, 0:HD],
                                rec,
                            )
            # ship this chunk's two token blocks to both batch slots (the
            # other-batch copy is zeroed so receivers just add both). All
            # high priority so the round's trigger fires as soon as possible
            # (gpsimd is unusable here: its TENSOR_SCALAR is ~8us/op).
            with tc.high_priority():
                for s in (J2 // 2, 4 + J2 // 2):
                    st = pr.tile(
                        [128, 2, HCOLS], BF16, tag=f"st{J2 % 2}",
                        name=f"st_{J2}_{s}"
                    )
                    nc.vector.tensor_scalar_mul(
                        st, attn_sb[:, 2 * J2 : 2 * J2 + 2, :], zm_sb[:, s : s + 1]
                    )
                    nc.sync.dma_start(
                        out=ain[s].rearrange("(t p) c -> p t c", p=128), in_=st
                    )

        # ---- attention: biggest chunk of each round first so the round's
        #      last (smallest) chunk has a tiny exp tail and the collective
        #      triggers right after the round's PE work ends ----
        k_sub(0)
        q_proj(2)
        v_pair(0)
        process_chunk(2, a2a_in1, hooks={
            1: [lambda: k_sub(1), lambda: v_pair(2)],
            2: [lambda: k_sub(2), lambda: v_pair(4)],
        })
        q_proj(4)
        process_chunk(4, a2a_in1, hooks={
            3: [lambda: k_sub(3), lambda: v_pair(6)],
            4: [lambda: k_sub(4), lambda: v_pair(8)],
        })
        q_proj(6)
        process_chunk(6, a2a_in1, hooks={
            5: [lambda: k_sub(5), lambda: v_pair(10)],
            6: [lambda: k_sub(6), lambda: v_pair(12)],
        })
        q_proj(0)
        process_chunk(0, a2a_in1)
        with tc.high_priority():
            nc.gpsimd.collective_compute(
                "AllToAll",
                mybir.AluOpType.bypass,
                replica_groups=[list(range(NCORES))],
                ins=[a2a_in1[:]],
                outs=[a2a_out1[:]],
            )

        k_sub(7)
        v_pair(14)
        q_proj(7)
        process_chunk(7, a2a_in2)
        q_proj(5)
        process_chunk(5, a2a_in2)
        q_proj(3)
        process_chunk(3, a2a_in2)
        q_proj(1)
        process_chunk(1, a2a_in2)
        with tc.high_priority():
            nc.gpsimd.collective_compute(
                "AllToAll",
                mybir.AluOpType.bypass,
                replica_groups=[list(range(NCORES))],
                ins=[a2a_in2[:]],
                outs=[a2a_out2[:]],
            )

        # ---- resident slice of w2 (h4 0-1) into the SBUF slots that die
        #      with the attention phase (exact-size fits, no slot growth);
        #      h4 2-7 are streamed per m2 pair ----
        # slow scalar DMA queue: not needed until m2's resident phase, and
        # this keeps the sync/gpsimd queues free for the w1/w2 streams
        w2resA = pb.tile([128, 4, D], BF16, tag="slotE")
        nc.scalar.dma_start(out=w2resA, in_=w2[:, 0, :, :])
        w2resB = pb.tile([128, 4, D], BF16, tag="slotF")
        nc.scalar.dma_start(out=w2resB, in_=w2[:, 1, :, :])

        # ---- recv + LN1 + transpose to hT, then m1 in token halves so the
        #      round-A half starts while round B's collective drains ----
        h_sb = pb.tile([128, NRB, D], F32, tag="slotD")  # reuses QT slot
        h_bf = pb.tile([128, NRB, D], BF16, tag="slotI")  # bf16 copy for hT
        res2 = pb.tile([128, NRB, D], F32, tag="slotC")  # reuses KT slot
        gT = pb.tile([128, 32, ROWS], BF16, tag="slotA")  # reuses xT slot

        def recv_adds(tb, aout, ti, rnd):
            # sync-issued DMAs (collective-completion deps enforced there).
            # The r0/r1 tiles share the round's SEND tile tag: the WAR
            # rotation pins every recv after that round's sends in the
            # static schedule, so recv/LN work can never block an engine
            # queue ahead of a not-yet-issued send (the scheduler's
            # collective-latency model is otherwise too optimistic).
            # Adds split gpsimd/vector by column group (disjoint res1
            # ranges) so each token block's chain uses both engines.
            for g in range(4):
                eng = nc.gpsimd if g < 2 else nc.vector
                r0 = pr.tile([128, HCOLS], BF16, tag=f"st{rnd}",
                             name=f"r0_{tb}_{g}")
                nc.sync.dma_start(
                    out=r0,
                    in_=aout[g].rearrange("(t p) c -> p t c", p=128)[:, ti, :],
                )
                r1 = pr.tile([128, HCOLS], BF16, tag=f"st{rnd}",
                             name=f"r1_{tb}_{g}")
                nc.sync.dma_start(
                    out=r1,
                    in_=aout[4 + g].rearrange("(t p) c -> p t c", p=128)[
                        :, ti, :
                    ],
                )
                # exactly one of the pair is nonzero (zmask), so the bf16
                # intermediate sum is exact
                ta = pr.tile([128, HCOLS], BF16, tag="ta", name=f"ta_{tb}_{g}")
                eng.tensor_add(ta, r0, r1)
                dst = res1[:, tb, g * HCOLS : (g + 1) * HCOLS]
                eng.tensor_add(dst, dst, ta)

        def ln_row(src_t, tb, out_ap, bf_ap=None):
            stats = ps.tile([128, 2, 6], F32, tag="stats")
            nc.vector.bn_stats(stats[:, 0, :], src_t[:, tb, 0:512])
            nc.vector.bn_stats(stats[:, 1, :], src_t[:, tb, 512:1024])
            mv = ps.tile([128, 2], F32, tag="mv")
            nc.vector.bn_aggr(mv, stats)
            std = ps.tile([128, 1], F32, tag="std")
            nc.scalar.activation(
                std, mv[:, 1:2], mybir.ActivationFunctionType.Sqrt,
                bias=eps_sb[:, 0:1], scale=1.0,
            )
            rstd = ps.tile([128, 1], F32, tag="rstd")
            nc.vector.reciprocal(rstd, std)
            # ln_g == 1, ln_b == 0 in this problem, so affine is identity
            nc.vector.tensor_scalar(
                out=out_ap,
                in0=src_t[:, tb, :],
                scalar1=mv[:, 0:1],
                scalar2=rstd,
                op0=mybir.AluOpType.subtract,
                op1=mybir.AluOpType.mult,
            )
            if bf_ap is not None:
                # bf16 shadow copy on the (idle-here) scalar engine
                nc.scalar.copy(bf_ap, out_ap)

        def ln_hT(tb):
            ln_row(res1, tb, h_sb[:, tb, :], h_bf[:, tb, :])
            for f4 in range(2):
                psT = pp.tile(
                    [128, 4, 128], BF16, tag="pu", name=f"psT_{tb}_{f4}"
                )
                for fs in range(4):
                    fc = 4 * f4 + fs
                    nc.tensor.transpose(
                        psT[:, fs, :],
                        h_bf[:, tb, fc * 128 : (fc + 1) * 128],
                        ident_b,
                    )
                # scalar engine: keeps the PSUM evacuation off the vector
                # queue, which carries LN + the odd round's send path here
                nc.scalar.copy(
                    hT[:, 4 * f4 : 4 * f4 + 4, tb * 128 : (tb + 1) * 128],
                    psT,
                )

        def m1_half(half):
            c0 = 256 * half
            for o4 in range(8):
                w1c = pws.tile(
                    [128, 8, 512], BF16, tag="w1c", name=f"w1c_{half}_{o4}"
                )
                # alternate DMA queues: one queue (~155 GB/s) is below the
                # ~240 GB/s the m1 matmuls consume weights at
                weng = nc.sync if o4 % 2 == 0 else nc.gpsimd
                weng.dma_start(out=w1c, in_=w1[:, o4, :, :])
                for os_ in range(4):
                    oc = o4 * 4 + os_
                    psm = pp.tile([128, 256], F32, tag="pqv", name=f"psm_{half}_{oc}")
                    for ic in range(8):
                        nc.tensor.matmul(
                            psm,
                            w1c[:, ic, os_ * 128 : (os_ + 1) * 128],
                            hT[:, ic, c0 : c0 + 256],
                            start=(ic == 0),
                            stop=(ic == 7),
                        )
                    nc.scalar.activation(
                        gT[:, oc, c0 : c0 + 256], psm,
                        mybir.ActivationFunctionType.Gelu,
                        bias=b1_sb[:, oc : oc + 1], scale=1.0,
                    )

        def m2_pair(tbp):
            tbs = (2 * tbp, 2 * tbp + 1)
            pso = {
                tb: pp.tile([128, 2, 512], F32, tag="ps", name=f"pso_{tb}")
                for tb in tbs
            }
            # streamed phase: h4 2..7, each w2c shared by both tbs
            for h4 in range(2, 8):
                w2c = pws.tile(
                    [128, 4, D], BF16, tag="w2c", name=f"w2c_{tbp}_{h4}"
                )
                weng = nc.gpsimd if h4 % 2 == 0 else nc.sync
                weng.dma_start(out=w2c, in_=w2[:, h4, :, :])
                for hs in range(4):
                    for tb in tbs:
                        for f2 in range(2):
                            nc.tensor.matmul(
                                pso[tb][:, f2, :],
                                gT[:, 4 * h4 + hs, tb * 128 : (tb + 1) * 128],
                                w2c[:, hs, f2 * 512 : (f2 + 1) * 512],
                                start=(h4 == 2 and hs == 0),
                                stop=False,
                            )
            # resident phase per tb (h4 0..1), staggered so the first tb's
            # evacuate/LN/DMA overlaps the second tb's matmuls
            for tb in tbs:
                for h4 in range(2):
                    wr = w2resA if h4 == 0 else w2resB
                    for hs in range(4):
                        for f2 in range(2):
                            nc.tensor.matmul(
                                pso[tb][:, f2, :],
                                gT[:, 4 * h4 + hs, tb * 128 : (tb + 1) * 128],
                                wr[:, hs, f2 * 512 : (f2 + 1) * 512],
                                start=False,
                                stop=(h4 == 1 and hs == 3),
                            )
                # b2 == 0 in this problem (skipped)
                nc.vector.tensor_add(
                    res2[:, tb, :],
                    pso[tb].rearrange("p a b -> p (a b)"),
                    h_sb[:, tb, :],
                )
                o_t = ps.tile([128, D], F32, tag="o_t", bufs=2)
                ln_row(res2, tb, o_t)
                nc.sync.dma_start(out=out[tb * 128 : (tb + 1) * 128, :], in_=o_t)

        # half 0 (round A rows) first; round B's recv + LN overlap m1/m2
        # of half 0, so the PE never waits on the second collective.
        recv_adds(0, a2a_out1, 0, 0)
        recv_adds(1, a2a_out1, 1, 0)
        ln_hT(0)
        ln_hT(1)
        m1_half(0)
        recv_adds(2, a2a_out2, 0, 1)
        recv_adds(3, a2a_out2, 1, 1)
        ln_hT(2)
        ln_hT(3)
        m2_pair(0)
        m1_half(1)
        m2_pair(1)

    nc.compile()
    return nc


_NC_CACHE = [None]


def kernel(**inputs) -> np.ndarray:
    import ml_dtypes

    x = np.asarray(inputs["x"], np.float32)
    wq = np.asarray(inputs["wq"], np.float32)
    wk = np.asarray(inputs["wk"], np.float32)
    wv = np.asarray(inputs["wv"], np.float32)
    w1 = np.asarray(inputs["w1"], np.float32)
    b1 = np.asarray(inputs["b1"], np.float32)
    w2 = np.asarray(inputs["w2"], np.float32)

    # The kernel folds these away; setup_inputs() constructs them as
    # zeros/ones. Fail loudly if that ever changes.
    for nm in ("bq", "bk", "bv", "b2"):
        if nm in inputs:
            assert not np.any(np.asarray(inputs[nm])), f"{nm} expected zero"
    if "ln_b" in inputs:
        assert not np.any(np.asarray(inputs["ln_b"])), "ln_b expected zero"
    if "ln_g" in inputs:
        assert np.all(np.asarray(inputs["ln_g"]) == 1.0), "ln_g expected ones"

    if _NC_CACHE[0] is None:
        _NC_CACHE[0] = _build()
    nc = _NC_CACHE[0]

    bf = ml_dtypes.bfloat16

    def pmaj_in(m):  # [D, cols] -> [p, ic, cols] partition-major
        return np.ascontiguousarray(
            m.reshape(8, 128, m.shape[1]).transpose(1, 0, 2)
        ).astype(bf)

    mask = np.triu(np.ones((128, 128), np.float32))
    # w1 [1024, 4096] -> [p, o4, ic, 512]; w2 [4096, 1024] -> [p, h4, hs, 1024]
    w1b = np.ascontiguousarray(
        w1.reshape(8, 128, 8, 512).transpose(1, 2, 0, 3)
    ).astype(bf)
    w2b = np.ascontiguousarray(
        w2.reshape(8, 4, 128, D).transpose(2, 0, 1, 3)
    ).astype(bf)
    # x[b].T -> [p, tq, ic, 512]
    xT_b = [
        np.ascontiguousarray(
            x[b].T.reshape(8, 128, 4, 512).transpose(1, 2, 0, 3)
        ).astype(bf)
        for b in range(B)
    ]
    in_maps = []
    for c in range(NCORES):
        b, q = c // 4, c % 4
        cols = slice(HCOLS * q, HCOLS * (q + 1))
        rows = slice(ROWS * q, ROWS * (q + 1))
        zm = np.zeros(NCORES, np.float32)
        zm[4 * b : 4 * b + 4] = 1.0
        in_maps.append(
            {
                "xbT": xT_b[b],
                "xr": np.ascontiguousarray(x[b, rows]),
                "wq_c": pmaj_in(np.ascontiguousarray(wq[:, cols]) * 0.125),
                "wk_c": pmaj_in(np.ascontiguousarray(wk[:, cols])),
                "wv_c": pmaj_in(np.ascontiguousarray(wv[:, cols])),
                "w1": w1b,
                "b1": b1,
                "w2": w2b,
                "mask_tri": mask,
                "zmask": zm,
            }
        )

    res = run_bass_kernel_spmd(nc, in_maps, list(range(NCORES)))
    outp = np.empty((B, L, D), np.float32)
    for c in range(NCORES):
        b, q = c // 4, c % 4
        outp[b, ROWS * q : ROWS * (q + 1)] = res.results[c]["out"]
    if getattr(res, "exec_time_ns", None) is not None:
        kernel.last_exec_time_ns = res.exec_time_ns
    return outp


kernel.last_exec_time_ns = None
